# revision 52
# baseline (speedup 1.0000x reference)
"""MoE FFN (BertGeneration-style) on 8 TRN2 NeuronCores, expert-parallel.

Problem: 8192 tokens, expert = task_id % 8, per-expert FFN
(768 -> 3072 gelu -> 768) + residual + per-expert LayerNorm.

Strategy: routing (dispatch/combine) is a host-side permutation; each of the
8 cores runs one expert's FFN over its 1024-token block.  Both GEMMs run in
fp8 e4m3 with MatmulPerfMode.DoubleRow (2 k-tiles per instruction, 2x the
bf16/fp32r rate: ~110ns per 256-col matmul vs 222ns).  Accuracy envelope:
fp8 quantization of x/W1/h/W2 gives rel_err ~1.05e-2 end-to-end (measured
against the fp32 reference; tolerance 2e-2) because the residual + LayerNorm
dilute the FFN error (|y| << |x|).

Scaling scheme (all powers of two, exact in fp32):
  xq = fp8(x*SX), w1q = fp8(W1*SW1)  -> GEMM1 psum = u*(SX*SW1)
  h8 = fp8(gelu(psum/(SX*SW1) + b1))  (activation pre-scale, fp8 output)
  w2q = fp8(W2*SW2)                  -> GEMM2 psum = y*SW2
  residual xn = (x + b2)*SW2; LayerNorm(z*SW2) == LayerNorm(z) exactly
  (LN is scale-invariant), so no descale op is ever needed.

On-chip phases:
  phase 1:  hT[i, m] = gelu(W1.T @ xT + b1)   (h transposed, fp8 in SBUF)
  phase 2:  y[m, h]  = hT.T @ W2; z = y + xn; LayerNorm along h on
            DVE (stats) + Scalar (normalize), stored in halves.
"""

import sys

if "/opt/trn_rl_repo" not in sys.path:
    sys.path.insert(0, "/opt/trn_rl_repo")

import numpy as np
import ml_dtypes


def _install_axon_hooks_shim():
    """Provide antenv.axon_hooks (NTFF profiling hook) when the image's
    antenv lacks it — a thin ctypes wrapper over libaxon_pjrt.so, matching
    trn_agent_boot.trn_boot._ntff_profile_via_ctypes.  Only exercised when
    profiling is requested (BASS_TRACE); harmless otherwise."""
    import contextlib
    import ctypes
    import types

    try:
        import antenv.axon_hooks  # noqa: F401
        return
    except ImportError:
        pass
    try:
        import antenv
    except ImportError:
        return

    mod = types.ModuleType("antenv.axon_hooks")
    _state = {"hook": None, "init": False}

    def set_axon_ntff_profile_hook(h):
        _state["hook"] = h
        _state["init"] = True

    def get_axon_ntff_profile_hook():
        if _state["init"]:
            return _state["hook"]
        _state["init"] = True
        try:
            lib = ctypes.CDLL("/opt/axon/libaxon_pjrt.so")
        except OSError:
            return None
        if not hasattr(lib, "axon_start_nrt_profile"):
            return None
        lib.axon_start_nrt_profile.argtypes = [
            ctypes.POINTER(ctypes.c_int64), ctypes.c_size_t]
        lib.axon_start_nrt_profile.restype = ctypes.c_int64
        lib.axon_stop_nrt_profile.argtypes = [ctypes.c_char_p]
        lib.axon_stop_nrt_profile.restype = ctypes.c_int64

        @contextlib.contextmanager
        def _hook(output_dir, device_ids):
            import jax
            jax.devices()
            if device_ids:
                ids = (ctypes.c_int64 * len(device_ids))(*device_ids)
                rc = lib.axon_start_nrt_profile(ids, len(device_ids))
            else:
                rc = lib.axon_start_nrt_profile(None, 0)
            if rc != 0:
                raise RuntimeError(f"axon_start_nrt_profile rc={rc}")
            try:
                yield
            finally:
                n = lib.axon_stop_nrt_profile(str(output_dir).encode())
                print(f"profile: {n} file(s) written to {output_dir}")

        _state["hook"] = _hook
        return _hook

    mod.set_axon_ntff_profile_hook = set_axon_ntff_profile_hook
    mod.get_axon_ntff_profile_hook = get_axon_ntff_profile_hook
    sys.modules["antenv.axon_hooks"] = mod
    antenv.axon_hooks = mod


_install_axon_hooks_shim()

E = 8
N = 8192
H = 768
I = 3072
C = N // E        # 1024 tokens per expert/core
KT = H // 128     # 6   k-tiles (hidden dim)
KP = KT // 2      # 3   k-pairs for DoubleRow
IT = I // 128     # 24  i-tiles (intermediate dim)
IP = IT // 2      # 12  i-pairs for DoubleRow
MT = C // 128     # 8   m-tiles (token dim per core)
EPS = 1e-12

SX = 32.0         # x scale   (|x|max*SX = 167 < 240 e4m3 max)
SW1 = 4096.0      # W1 scale  (keeps |W1| quanta out of fp8 denormals)
SW2 = 8192.0      # W2 scale
DESCALE1 = 1.0 / (SX * SW1)

_CACHE = {}
F8 = ml_dtypes.float8_e4m3


def _build_nc(act_name="Gelu"):
    from contextlib import ExitStack

    import concourse.tile as tile
    from concourse import bacc, mybir

    f32 = mybir.dt.float32
    f8 = mybir.dt.float8e4
    AF = mybir.ActivationFunctionType
    act_fn = getattr(AF, act_name)
    ALU = mybir.AluOpType
    DR = mybir.MatmulPerfMode.DoubleRow

    nc = bacc.Bacc("TRN2", target_bir_lowering=False, debug=False, num_devices=8)

    # the kp=0 pair ships as two separate contiguous tensors (a small leader
    # chunk + the rest) so the startup DMAs are dense per-partition reads —
    # column-slicing one big xT tensor yields strided descriptors that take
    # ~4us to land and stall the first matmuls
    xTa = nc.dram_tensor("xTa", [128, 2, 256], f8, kind="ExternalInput").ap()
    xTb = nc.dram_tensor("xTb", [128, 2, C - 256], f8, kind="ExternalInput").ap()
    xT = nc.dram_tensor("xT", [128, KT - 2, C], f8, kind="ExternalInput").ap()
    w1 = nc.dram_tensor("w1", [128, IT, KT, 128], f8, kind="ExternalInput").ap()
    w2 = nc.dram_tensor("w2", [128, IP, 2, H], f8, kind="ExternalInput").ap()
    b1t = nc.dram_tensor("b1t", [128, IT], f32, kind="ExternalInput").ap()
    xn = nc.dram_tensor("xn", [128, MT, H], f32, kind="ExternalInput").ap()
    out = nc.dram_tensor("out", [128, MT, H], f32, kind="ExternalOutput").ap()

    with ExitStack() as ctx:
        tc = ctx.enter_context(tile.TileContext(nc))
        persist = ctx.enter_context(tc.tile_pool(name="persist", bufs=1))
        psA = ctx.enter_context(tc.tile_pool(name="psA", bufs=2, space="PSUM"))
        psB = ctx.enter_context(tc.tile_pool(name="psB", bufs=2, space="PSUM"))
        w1pool = ctx.enter_context(tc.tile_pool(name="w1s", bufs=5))
        zpool = ctx.enter_context(tc.tile_pool(name="zs", bufs=3))
        zqpool = ctx.enter_context(tc.tile_pool(name="zq", bufs=2))
        spool = ctx.enter_context(tc.tile_pool(name="small", bufs=4))

        # hT split in two so GEMM2's early i-pairs don't wait on the last
        # GEMM1 activations (phase-boundary bubble)
        hTa = persist.tile([128, IT // 2, C], f8, name="hTa")
        hTb = persist.tile([128, IT // 2, C], f8, name="hTb")
        w2s = [persist.tile([128, 3, 2, H], f8, name=f"w2s{t}") for t in range(4)]
        # xk0 split so the very first matmul only waits on a 512B/partition
        # leader chunk instead of the whole 2KB pair tile
        xk0a = persist.tile([128, 2, 256], f8, name="xk0a")
        xk0b = persist.tile([128, 2, C - 256], f8, name="xk0b")
        xk = [None] + [persist.tile([128, 2, C], f8, name=f"xk{k}", tag=f"xk{k}")
                       for k in range(1, KP)]
        # residual (x + b2)*SW2 staged in two bulk tiles (2 DMAs, pinned
        # behind early GEMM1 output so they stay out of the startup window)
        xnA = persist.tile([128, 4, H], f32, name="xnA")
        xnB = persist.tile([128, 4, H], f32, name="xnB")
        b1s = persist.tile([128, IT], f32, name="b1s")
        epsT = persist.tile([128, 1], f32, name="epsT")
        warm = persist.tile([128, 2, 256], f8, name="warm")

        nc.vector.memset(epsT, EPS)
        nc.vector.memset(warm.bitcast(mybir.dt.uint8), 0)

        # ---- phase 1: hT = gelu((W1q.T @ xq) / (SX*SW1) + b1), fp8 out ----
        # startup: the smallest PE-critical transfers lead both queues so
        # the first matmul fires as early as possible; w1 streams as it-pair
        # DMAs (halves the sync-queue issue load vs per-it DMAs)
        w1a0 = persist.tile([128, KT, 128], f8, name="w1a0")
        w1a1 = persist.tile([128, KT, 128], f8, name="w1a1")
        nc.sync.dma_start(out=w1a0, in_=w1[:, 0])
        nc.sync.dma_start(out=w1a1, in_=w1[:, 1])
        nc.scalar.dma_start(out=xk0a, in_=xTa)
        nc.scalar.dma_start(out=b1s, in_=b1t)
        nc.gpsimd.dma_start(out=xk0b, in_=xTb)
        nc.gpsimd.dma_start(out=xk[1], in_=xT[:, 0:2])
        nc.gpsimd.dma_start(out=xk[2], in_=xT[:, 2:4])

        # warm-up: dummy fp8 matmuls on zeroed SBUF while the startup DMAs
        # are in flight, so the PE clock is fully ramped (the hw p-state
        # needs ~3us of continuous activity) when the real GEMM1 begins
        wp = psB.tile([128, H], f32, name="py", tag="py")
        for wi in range(10):
            nc.tensor.matmul(
                wp[:, 0:256], lhsT=warm[:, :, 0:128], rhs=warm,
                start=True, stop=True, perf_mode=DR, skip_group_check=True)

        w1p = None
        for it in range(IT):
            if it == 0:
                w1t = w1a0
            elif it == 1:
                w1t = w1a1
            else:
                if (it - 2) % 2 == 0:
                    w1p = w1pool.tile([128, 2, KT, 128], f8, name="w1p",
                                      tag="w1p")
                    nc.sync.dma_start(out=w1p, in_=w1[:, it:it + 2])
                w1t = w1p[:, (it - 2) % 2]
            ph = psA.tile([128, C], f32, name="ph", tag="ph")
            # psum start/stop are per 2KB bank (the hw "zero region"): only
            # the first touch of each bank starts; later first-touches of
            # other chunks auto-zero via the pending-zero mechanism
            for kp in range(KP):
                lhsT = w1t[:, 2 * kp:2 * kp + 2, :]
                for cc in range(4):
                    if kp == 0:
                        rhs = xk0a[:, :, :] if cc == 0 else \
                            xk0b[:, :, (cc - 1) * 256:cc * 256]
                    else:
                        rhs = xk[kp][:, :, cc * 256:(cc + 1) * 256]
                    nc.tensor.matmul(
                        ph[:, cc * 256:(cc + 1) * 256],
                        lhsT=lhsT,
                        rhs=rhs,
                        start=(kp == 0 and cc in (0, 2)),
                        stop=(kp == KP - 1 and cc in (1, 3)),
                        perf_mode=DR,
                        skip_group_check=True,
                    )
            dst = hTa[:, it, :] if it < IT // 2 else hTb[:, it - IT // 2, :]
            nc.scalar.activation(dst, ph, act_fn, bias=b1s[:, it:it + 1],
                                 scale=DESCALE1)
            # W2 + residual prefetch pinned behind GEMM1 progress (dummy copy
            # creates the dep) so the scheduler can't hoist them into the
            # startup window
            if it in (1, 3, 5, 7):
                t = (1, 3, 5, 7).index(it)
                nc.vector.tensor_copy(
                    out=w2s[t][:, 0, 0, 0:4].bitcast(f32),
                    in_=hTa[:, it, 0:4].bitcast(f32))
                nc.sync.dma_start(out=w2s[t], in_=w2[:, 3 * t:3 * t + 3])
            elif it == 16:
                # residual loads run late in phase 1, after the w1 stream has
                # fully issued — their 12KB/partition transfers otherwise
                # starve the w1 stream and stall GEMM1 (PE drops out of its
                # ramped p-state, halving matmul throughput)
                nc.vector.tensor_copy(out=xnA[:, 0, 0:1],
                                      in_=hTb[:, 4, 0:4].bitcast(f32))
                nc.gpsimd.dma_start(out=xnA, in_=xn[:, 0:4])
            elif it == 20:
                nc.vector.tensor_copy(out=xnB[:, 0, 0:1],
                                      in_=hTb[:, 8, 0:4].bitcast(f32))
                nc.gpsimd.dma_start(out=xnB, in_=xn[:, 4:8])

        # ---- phase 2: y = hT.T @ W2q; z = y + xn; LayerNorm ----
        def lhsT_for(ip, ms):
            if ip < IP // 2:
                return hTa[:, 2 * ip:2 * ip + 2, ms]
            jp = ip - IP // 2
            return hTb[:, 2 * jp:2 * jp + 2, ms]

        def epilogue_tail(mt, zt, sums):
            """sums[:, 0:1] = sum(z), sums[:, 1:2] = sum(z^2) over H, from the
            accum_out of the residual add (scalar_tensor_tensor) and a Scalar
            Square pass — replaces the 3x bn_stats + bn_aggr DVE chain.
            var = E[z^2] - mu^2 is safe: mu^2 << E[z^2] for this data."""
            mv = spool.tile([128, 2], f32, name="mv", tag="mv")
            msq = spool.tile([128, 1], f32, name="msq", tag="msq")
            nc.vector.tensor_scalar(
                out=mv[:, 0:1], in0=sums[:, 0:1], scalar1=1.0 / H,
                scalar2=None, op0=ALU.mult)
            nc.vector.tensor_scalar(
                out=msq, in0=mv[:, 0:1], scalar1=mv[:, 0:1],
                scalar2=None, op0=ALU.mult)
            nc.vector.scalar_tensor_tensor(
                out=mv[:, 1:2], in0=sums[:, 1:2], scalar=1.0 / H, in1=msq,
                op0=ALU.mult, op1=ALU.subtract)
            rstd = spool.tile([128, 1], f32, name="rstd", tag="rstd")
            nc.scalar.activation(rstd, mv[:, 1:2], AF.Sqrt, bias=epsT)
            nc.vector.reciprocal(out=rstd, in_=rstd)
            # normalize split across DVE (half 0) and Scalar (half 1): DVE is
            # the phase-2 critical engine (~3.5us/mt vs PE 3.9) — without the
            # offload its epilogue backlog serializes into a long kernel tail
            nb = spool.tile([128, 1], f32, name="nb", tag="nb")
            nc.vector.tensor_scalar(
                out=nb, in0=mv[:, 0:1], scalar1=rstd, scalar2=-1.0,
                op0=ALU.mult, op1=ALU.mult)
            h2 = H // 2
            nc.vector.tensor_scalar(
                out=zt[:, 0:h2], in0=zt[:, 0:h2], scalar1=mv[:, 0:1],
                scalar2=rstd, op0=ALU.subtract, op1=ALU.mult)
            nc.sync.dma_start(out=out[:, mt, 0:h2], in_=zt[:, 0:h2])
            nc.scalar.activation(zt[:, h2:H], zt[:, h2:H], AF.Identity,
                                 bias=nb, scale=rstd)
            nc.gpsimd.dma_start(out=out[:, mt, h2:H], in_=zt[:, h2:H])

        for mt in range(MT - 1):
            py = psB.tile([128, H], f32, name="py", tag="py")
            ms = slice(mt * 128, (mt + 1) * 128)
            for ip in range(IP):
                lhsT = lhsT_for(ip, ms)
                for hc in range(3):
                    nc.tensor.matmul(
                        py[:, hc * 256:(hc + 1) * 256],
                        lhsT=lhsT,
                        rhs=w2s[ip // 3][:, ip % 3, :, hc * 256:(hc + 1) * 256],
                        start=(ip == 0 and hc in (0, 2)),
                        stop=(ip == IP - 1 and hc in (1, 2)),
                        perf_mode=DR,
                        skip_group_check=True,
                    )
            # z = y*SW2 + (x + b2)*SW2; LayerNorm(z*SW2) == LayerNorm(z)
            xnsrc = xnA if mt < 4 else xnB
            zt = zpool.tile([128, H], f32, name="zt", tag="zt")
            sums = spool.tile([128, 2], f32, name="sums", tag="sums")
            nc.vector.scalar_tensor_tensor(
                out=zt, in0=py[:, 0:H], scalar=1.0, in1=xnsrc[:, mt % 4],
                op0=ALU.mult, op1=ALU.add, accum_out=sums[:, 0:1])
            zsq = zqpool.tile([128, H], f32, name="zsq", tag="zsq")
            nc.scalar.activation(zsq, zt, AF.Square, accum_out=sums[:, 1:2])
            epilogue_tail(mt, zt, sums)

        # last mt runs as two column blocks into two (now free) phase-1 psum
        # tiles, so the residual add + stats for columns 0:512 overlap the
        # final 512:768 matmuls — shortens the kernel's serial tail
        mt = MT - 1
        ms = slice(mt * 128, (mt + 1) * 128)
        pyA = psA.tile([128, C], f32, name="pyA", tag="ph")
        pyB = psA.tile([128, C], f32, name="pyB", tag="ph")
        for ip in range(IP):
            lhsT = lhsT_for(ip, ms)
            for hc in range(2):
                nc.tensor.matmul(
                    pyA[:, hc * 256:(hc + 1) * 256],
                    lhsT=lhsT,
                    rhs=w2s[ip // 3][:, ip % 3, :, hc * 256:(hc + 1) * 256],
                    start=(ip == 0 and hc == 0),
                    stop=(ip == IP - 1 and hc == 1),
                    perf_mode=DR,
                    skip_group_check=True,
                )
        zt = zpool.tile([128, H], f32, name="zt", tag="zt")
        zsq = zqpool.tile([128, H], f32, name="zsq", tag="zsq")
        pp = spool.tile([128, 4], f32, name="pp", tag="pp")
        nc.vector.scalar_tensor_tensor(
            out=zt[:, 0:512], in0=pyA[:, 0:512], scalar=1.0,
            in1=xnB[:, 3, 0:512], op0=ALU.mult, op1=ALU.add,
            accum_out=pp[:, 0:1])
        nc.scalar.activation(zsq[:, 0:512], zt[:, 0:512], AF.Square,
                             accum_out=pp[:, 1:2])
        for ip in range(IP):
            nc.tensor.matmul(
                pyB[:, 0:256],
                lhsT=lhsT_for(ip, ms),
                rhs=w2s[ip // 3][:, ip % 3, :, 512:768],
                start=(ip == 0),
                stop=(ip == IP - 1),
                perf_mode=DR,
                skip_group_check=True,
            )
        nc.vector.scalar_tensor_tensor(
            out=zt[:, 512:768], in0=pyB[:, 0:256], scalar=1.0,
            in1=xnB[:, 3, 512:768], op0=ALU.mult, op1=ALU.add,
            accum_out=pp[:, 2:3])
        nc.scalar.activation(zsq[:, 512:768], zt[:, 512:768], AF.Square,
                             accum_out=pp[:, 3:4])
        sums = spool.tile([128, 2], f32, name="sums", tag="sums")
        nc.vector.tensor_add(sums[:, 0:1], pp[:, 0:1], pp[:, 2:3])
        nc.vector.tensor_add(sums[:, 1:2], pp[:, 1:2], pp[:, 3:4])
        epilogue_tail(mt, zt, sums)

    nc.compile()
    return nc


def _get_nc(act_name="Gelu"):
    key = ("nc", act_name)
    if key not in _CACHE:
        _CACHE[key] = _build_nc(act_name)
    return _CACHE[key]


def _shard_inputs(x, task_ids, W1, b1, W2, b2):
    """Host-side dispatch: stable-sort tokens by expert id, chunk into E
    equal capacity-C blocks (exactly the reference's xs = x[order].reshape),
    quantize weights/activations to fp8 e4m3 with power-of-2 scales."""
    expert = (task_ids.astype(np.int64) % E).astype(np.int32)
    order = np.argsort(expert, kind="stable")
    xs = x[order]
    in_maps = []
    for e in range(E):
        xe = xs[e * C:(e + 1) * C]                       # [C, H]
        xq = (xe * SX).astype(F8)                       # [C, H] fp8
        xTfull = xq.T.reshape(KT, 128, C).transpose(1, 0, 2)
        xTa = xTfull[:, 0:2, 0:256]
        xTb = xTfull[:, 0:2, 256:C]
        xT = xTfull[:, 2:KT]
        w1q = (W1[e] * SW1).astype(F8)                  # [H, I] fp8
        w1m = w1q.reshape(KT, 128, IT, 128).transpose(1, 2, 0, 3)
        w2q = (W2[e] * SW2).astype(F8)                  # [I, H] fp8
        w2m = w2q.reshape(IP, 2, 128, H).transpose(2, 0, 1, 3)
        b1t = b1[e].reshape(IT, 128).T
        xnm = ((xe + b2[e][None, :]) * SW2).reshape(MT, 128, H).transpose(1, 0, 2)
        in_maps.append({
            "xTa": np.ascontiguousarray(xTa),
            "xTb": np.ascontiguousarray(xTb),
            "xT": np.ascontiguousarray(xT),
            "w1": np.ascontiguousarray(w1m),
            "w2": np.ascontiguousarray(w2m),
            "b1t": np.ascontiguousarray(b1t, dtype=np.float32),
            "xn": np.ascontiguousarray(xnm, dtype=np.float32),
        })
    return in_maps, order


def kernel(x, task_ids, W1, b1, W2, b2, gamma, beta):
    from concourse import bass_utils

    x = np.asarray(x, dtype=np.float32)
    task_ids = np.asarray(task_ids)
    W1 = np.asarray(W1, dtype=np.float32)
    b1 = np.asarray(b1, dtype=np.float32)
    W2 = np.asarray(W2, dtype=np.float32)
    b2 = np.asarray(b2, dtype=np.float32)
    gamma = np.asarray(gamma, dtype=np.float32)
    beta = np.asarray(beta, dtype=np.float32)

    in_maps, order = _shard_inputs(x, task_ids, W1, b1, W2, b2)
    nc = _get_nc()
    res = bass_utils.run_bass_kernel_spmd(nc, in_maps, core_ids=list(range(E)))
    _CACHE["last_results"] = res

    z = np.concatenate(
        [res.results[e]["out"].transpose(1, 0, 2).reshape(C, H) for e in range(E)],
        axis=0)
    # per-expert gamma/beta (identity for this problem's inputs; applied on
    # host only when nontrivial, matching the reference's z*gamma + beta)
    if not (np.all(gamma == 1.0) and np.all(beta == 0.0)):
        blk = np.repeat(np.arange(E), C)  # reference uses capacity blocks
        z = z * gamma[blk] + beta[blk]
    out = np.empty((N, H), dtype=np.float32)
    out[order] = z
    return out
